# revision 7
# baseline (speedup 1.0000x reference)
"""Trainium2 Bass kernel for nn_CNNStateEncoder (dense_cnn).

Network per row (B*S rows, 8 features each):
  conv1 2x2 on [1,2,4] -> 32ch x [1,3]   == h1[96]  = A1[96,8]  @ x[8],  relu(+b1)
  conv2 1x2 on [32,1,3] -> 32ch x [1,2]  == h2[64]  = A2[64,96] @ h1,    relu(+b2)
  linear 64->64                          == out[64] = Wp[64,64] @ h2 + bp

Per-core mapping (data parallel, 65536 rows/core, feature-major, fp16 I/O).
PE on this part is pinned at 1.2GHz (HAM never releases), so the kernel is
tensor-bound: minimize PE column-slots via maximum tile packing.

  - host pre-transposes x to xT8 [8, 65536] fp16; rows split into four
    16384-row quarter-streams (PE row strips q=0..3)
  - 2048-row tiles; psum: h1 4 banks + h2 2 + out 2 = all 8
  - conv1: 4-way row-strip-packed K=8 matmuls N=512 (one bank each)
  - relu1+b1 split: DVE tensor_scalar on banks 0-1 || ACT on banks 2-3
  - conv2: K=96 col-packed pairs (0,0)/(0,64), 2 rounds of N=512
  - relu2+b2: one ACT op [128, 1024]
  - linear: Wp stationary, 4 disjoint rectangles (0,0)(64,64)(0,64)(64,0)
    -> ONE round of 4 concurrent N=512 matmuls
  - out+bp: one DVE tensor_scalar [128, 1024]
  - software-pipelined: PE issues conv1(t), conv2(t-1), linear(t-2) so it
    never head-of-line blocks; 1MB batched stores; host unpermutes
"""

import numpy as np

B, S, FEAT, OUT = 64, 8192, 8, 64
NCORES = 8
ROWS_TOTAL = B * S
ROWS_CORE = ROWS_TOTAL // NCORES   # 65536
TILE = 2048                        # rows per psum tile
NT = ROWS_CORE // TILE             # 32 tiles
TPB = 8                            # tiles per dma batch
NB = NT // TPB                     # 4 batches
QCH = ROWS_CORE // 4               # 16384 rows per strip-block

F16 = np.float16

# ---------------------------------------------------------------------------
# numpy-side packing
# ---------------------------------------------------------------------------

def pack_weights(W1, b1, W2, b2, Wp, bp):
    W1 = np.asarray(W1, np.float32)
    W2 = np.asarray(W2, np.float32)
    Wp = np.asarray(Wp, np.float32)
    b1 = np.asarray(b1, np.float32)
    b2 = np.asarray(b2, np.float32)
    bp = np.asarray(bp, np.float32)

    # A1 [96, 8]: h1[o*3+j] = sum_{kh,kw} x[kh*4 + j + kw] * W1[o,0,kh,kw]
    A1 = np.zeros((96, 8), np.float32)
    for o in range(32):
        for j in range(3):
            for kh in range(2):
                for kw in range(2):
                    A1[o * 3 + j, kh * 4 + j + kw] += W1[o, 0, kh, kw]
    b1_96 = np.repeat(b1, 3).astype(np.float32)

    # A2 [64, 96]: h2[c*2+w] = sum_{i,kw} h1[i*3 + w + kw] * W2[c,i,0,kw]
    A2 = np.zeros((64, 96), np.float32)
    for c in range(32):
        for w in range(2):
            for i in range(32):
                for kw in range(2):
                    A2[c * 2 + w, i * 3 + w + kw] += W2[c, i, 0, kw]
    b2_64 = np.repeat(b2, 2).astype(np.float32)

    a1t = np.zeros((128, 96), F16)
    for q in range(4):
        a1t[32 * q:32 * q + 8, :] = A1.T.astype(F16)
    a2t = A2.T.astype(F16)                      # [96, 64]
    wpt = np.zeros((128, 64), F16)
    wpt[0:64, :] = Wp.T.astype(F16)
    wpt[64:128, :] = Wp.T.astype(F16)
    b1c = b1_96.reshape(96, 1)
    b2c = np.concatenate([b2_64, b2_64]).reshape(128, 1)
    bpc = np.concatenate([bp, bp]).reshape(128, 1)
    return {"a1t": a1t, "a2t": a2t, "wpt": wpt,
            "b1c": b1c, "b2c": b2c, "bpc": bpc}


def build_in_maps(x, W1, b1, W2, b2, Wp, bp):
    x = np.ascontiguousarray(np.asarray(x, np.float32)).reshape(ROWS_TOTAL, FEAT)
    consts = pack_weights(W1, b1, W2, b2, Wp, bp)
    in_maps = []
    for c in range(NCORES):
        xc = x[c * ROWS_CORE:(c + 1) * ROWS_CORE]
        m = dict(consts)
        m["xT8"] = np.ascontiguousarray(xc.T.astype(F16))  # [8, 65536]
        in_maps.append(m)
    return in_maps


def reconstruct(results):
    outs = []
    for r in results:
        od = np.asarray(r["out"]).astype(np.float32)    # [128, 32768] fp16
        a = od.reshape(2, 64, NT, 2, 512)               # (h, f, t, g, i)
        res = np.empty((4, NT, 512, 64), np.float32)    # (block, t, i, f)
        res[0] = a[0, :, :, 0, :].transpose(1, 2, 0)
        res[1] = a[1, :, :, 0, :].transpose(1, 2, 0)
        res[2] = a[1, :, :, 1, :].transpose(1, 2, 0)
        res[3] = a[0, :, :, 1, :].transpose(1, 2, 0)
        outs.append(res.reshape(ROWS_CORE, OUT))        # row = 16384b+512t+i
    return np.concatenate(outs, 0).reshape(B, S, OUT)


# ---------------------------------------------------------------------------
# bass module
# ---------------------------------------------------------------------------

def build_nc():
    import concourse.bass as bass
    import concourse.bacc as bacc
    import concourse.mybir as mybir
    import concourse.tile as tile

    f32 = mybir.dt.float32
    f16 = mybir.dt.float16
    Relu = mybir.ActivationFunctionType.Relu
    Alu = mybir.AluOpType

    nc = bacc.Bacc(None, target_bir_lowering=False)

    xT8_d = nc.dram_tensor("xT8", [FEAT, ROWS_CORE], f16, kind="ExternalInput")
    a1t_d = nc.dram_tensor("a1t", [128, 96], f16, kind="ExternalInput")
    a2t_d = nc.dram_tensor("a2t", [96, 64], f16, kind="ExternalInput")
    wpt_d = nc.dram_tensor("wpt", [128, 64], f16, kind="ExternalInput")
    b1c_d = nc.dram_tensor("b1c", [96, 1], f32, kind="ExternalInput")
    b2c_d = nc.dram_tensor("b2c", [128, 1], f32, kind="ExternalInput")
    bpc_d = nc.dram_tensor("bpc", [128, 1], f32, kind="ExternalInput")
    out_d = nc.dram_tensor("out", [128, ROWS_CORE // 2], f16, kind="ExternalOutput")

    with tile.TileContext(nc) as tc:
        with (
            tc.tile_pool(name="consts", bufs=1) as cpool,
            tc.tile_pool(name="xin", bufs=2) as xpool,
            tc.tile_pool(name="h1s", bufs=3) as h1pool,
            tc.tile_pool(name="h2s", bufs=3) as h2pool,
            tc.tile_pool(name="osb", bufs=2) as opool,
            tc.tile_pool(name="ps_h1", bufs=1, space="PSUM") as ps_h1,
            tc.tile_pool(name="ps_h2", bufs=1, space="PSUM") as ps_h2,
            tc.tile_pool(name="ps_o", bufs=1, space="PSUM") as ps_o,
        ):
            a1t = cpool.tile([128, 96], f16)
            a2t = cpool.tile([96, 64], f16)
            wpt = cpool.tile([128, 64], f16)
            b1c = cpool.tile([96, 1], f32)
            b2c = cpool.tile([128, 1], f32)
            bpc = cpool.tile([128, 1], f32)
            nc.sync.dma_start(a1t[:], a1t_d[:])
            nc.sync.dma_start(a2t[:], a2t_d[:])
            nc.sync.dma_start(wpt[:], wpt_d[:])
            nc.sync.dma_start(b1c[:], b1c_d[:])
            nc.sync.dma_start(b2c[:], b2c_d[:])
            nc.sync.dma_start(bpc[:], bpc_d[:])

            def load_batch(T):
                xt = xpool.tile([128, TPB * 512], f16)
                for q in range(4):
                    nc.sync.dma_start(
                        xt[32 * q:32 * q + 8, :],
                        xT8_d[:, QCH * q + 4096 * T:QCH * q + 4096 * T + 4096],
                    )
                return xt

            xts = {0: load_batch(0)}
            h1s_t = {}
            h2s_t = {}
            outsb_cur = {}

            # software-pipelined: PE does conv1(t), conv2(t-1), linear(t-2)
            for t in range(NT + 2):
                if t < NT:
                    T, s = t // TPB, t % TPB
                    if s == 0 and T + 1 < NB and (T + 1) not in xts:
                        xts[T + 1] = load_batch(T + 1)
                    xt = xts[T]
                    # ---- conv1(t): 4-way strip-packed K=8, N=512 ----
                    h1ps = ps_h1.tile([96, 2048], f32)
                    for q in range(4):
                        nc.tensor.matmul(
                            h1ps[:, 512 * q:512 * q + 512],
                            a1t[32 * q:32 * q + 8, :],
                            xt[32 * q:32 * q + 8, 512 * s:512 * s + 512],
                            tile_position=(32 * q, 0),
                        )
                    # ---- relu1 + b1: DVE banks 0-1 || ACT banks 2-3 ----
                    h1s = h1pool.tile([96, 2048], f16)
                    nc.vector.tensor_scalar(
                        h1s[:, 0:1024], h1ps[:, 0:1024], b1c[:], 0.0,
                        Alu.add, Alu.max,
                    )
                    nc.scalar.activation(
                        h1s[:, 1024:2048], h1ps[:, 1024:2048], Relu, bias=b1c[:]
                    )
                    h1s_t[t] = h1s
                if 1 <= t <= NT:
                    u = t - 1
                    # ---- conv2(u): 2 rounds of col-packed K=96 pairs ----
                    h1s = h1s_t.pop(u)
                    h2ps = ps_h2.tile([128, 1024], f32)
                    for r in range(2):
                        nc.tensor.matmul(
                            h2ps[0:64, 512 * r:512 * r + 512],
                            a2t[:],
                            h1s[:, 1024 * r:1024 * r + 512],
                            tile_position=(0, 0),
                        )
                        nc.tensor.matmul(
                            h2ps[64:128, 512 * r:512 * r + 512],
                            a2t[:],
                            h1s[:, 1024 * r + 512:1024 * r + 1024],
                            tile_position=(0, 64),
                        )
                    # ---- relu2 + b2: one ACT op ----
                    h2s = h2pool.tile([128, 1024], f16)
                    nc.scalar.activation(h2s[:], h2ps[:], Relu, bias=b2c[:])
                    h2s_t[u] = h2s
                if 2 <= t:
                    u = t - 2
                    T, s = u // TPB, u % TPB
                    if s == 0:
                        outsb_cur[0] = opool.tile(
                            [128, TPB * 1024], f16, name="outsb"
                        )
                    outsb = outsb_cur[0]
                    # ---- linear(u): 4 disjoint rectangles, one round ----
                    h2s = h2s_t.pop(u)
                    outps = ps_o.tile([128, 1024], f32)
                    nc.tensor.matmul(
                        outps[0:64, 0:512], wpt[0:64, :], h2s[0:64, 0:512],
                        tile_position=(0, 0),
                    )
                    nc.tensor.matmul(
                        outps[64:128, 0:512], wpt[64:128, :], h2s[64:128, 0:512],
                        tile_position=(64, 64),
                    )
                    nc.tensor.matmul(
                        outps[64:128, 512:1024], wpt[0:64, :], h2s[0:64, 512:1024],
                        tile_position=(0, 64),
                    )
                    nc.tensor.matmul(
                        outps[0:64, 512:1024], wpt[64:128, :], h2s[64:128, 512:1024],
                        tile_position=(64, 0),
                    )
                    # ---- out + bp: one DVE op ----
                    nc.vector.tensor_scalar_add(
                        outsb[:, 1024 * s:1024 * s + 1024], outps[:], bpc[:]
                    )
                    if s == TPB - 1:
                        nc.sync.dma_start(
                            out_d[:, 8192 * T:8192 * T + 8192], outsb[:]
                        )

    nc.compile()
    return nc


# ---------------------------------------------------------------------------
# entry point
# ---------------------------------------------------------------------------

_CACHE = {}


def _get_nc():
    if "nc" not in _CACHE:
        _CACHE["nc"] = build_nc()
    return _CACHE["nc"]


def kernel(x, W1, b1, W2, b2, Wp, bp):
    from concourse.bass_utils import run_bass_kernel_spmd

    nc = _get_nc()
    in_maps = build_in_maps(x, W1, b1, W2, b2, Wp, bp)
    res = run_bass_kernel_spmd(nc, in_maps, core_ids=list(range(NCORES)))
    return reconstruct(res.results)
